# revision 18
# baseline (speedup 1.0000x reference)
"""Trainium2 Bass kernel for nn_RNN_6296422056099 — hybrid batch x time sharding.

RNN: xp = x @ W_ih.T + b_ih + b_hh ; h_t = tanh(xp_t + h_{t-1} @ W_hh.T)
out = softmax(relu(h @ W1.T + b1) @ W2.T + b2)

8 cores = bt batch-shards x tt time-shards. Each core runs T_local = S + W
recurrence steps over batch width w = B/bt, where the first W steps are
contraction warmup (tanh RNN forgets its initial state; W=64 reaches the
fp32 noise floor). Core (bi, ti=0) feeds real x[0:T_local] starting from the
true h=0, so ALL its outputs are exact; other cores discard the first W.

Per-core layout:
  - x is pre-transposed on host to [2, 100, T_local*w] so xp matmuls consume
    it directly (no on-chip transposes).
  - stage ring buffer [128, RB*w]: per t-block of w cols, partitions 0:64
    hold h_{t-1}^T, partitions 64:128 hold xp_t^T. Recurrence step = G
    matmuls with stationary [W_hh^T ; I_64] (identity-fold adds xp in the
    PE), then ACT tanh PSUM->stage.
  - phase 1 (xp production, lead chunks ahead) and phase 3 (MLP+softmax)
    interleave into the recurrence via an emission micro-queue.
"""

import sys

for p in ("/opt/trn_rl_repo",):
    if p not in sys.path:
        sys.path.append(p)

from contextlib import ExitStack

import numpy as np

import concourse.bass as bass
import concourse.tile as tile
from concourse import mybir
from concourse._compat import with_exitstack
from concourse.bacc import Bacc

F32 = mybir.dt.float32
AF = mybir.ActivationFunctionType

B, T, I, H = 128, 2048, 200, 64
NCORES = 8
IH = 100

# ---- sharding config ----
WIDTH = 128           # batch rows per core (w); bt = B//WIDTH, tt = NCORES//bt
WARM = 64             # contraction warmup steps
G = 2                 # recurrence groups per step

TRACE = False
LAST_EXEC_NS = None

XB_ENGINE = "gpsimd"     # queue for second-half x DMA
OUT_ENGINE = "scalar"    # queue for output DMA


def _cfg(w, T_local):
    ch = 512 // w                 # t-blocks per chunk (chunk = 512 (t,b) pairs)
    assert T_local % ch == 0
    nb = T_local // ch
    sl = (ch * w) // 128          # MM2 slices per chunk (=4)
    cpb = 512 // (4 * sl)         # chunks per psum-out bank (=32)
    return ch, nb, sl, cpb


@with_exitstack
def rnn_body(ctx: ExitStack, tc: tile.TileContext,
             xt, wfold, wih_a, wih_b, bias64, w1t, b1, w2te, out,
             w: int, T_local: int, lead: int, g2=(1.0, 1.0, 1.0, 1.0)):
    nc = tc.nc
    CH, NB, SL, CPB = _cfg(w, T_local)
    GW = w // G
    RC = lead + 3                 # ring size in chunks
    RB = RC * CH                  # ring size in t-blocks
    CW = CH * w                   # cols per chunk

    wpool = ctx.enter_context(tc.tile_pool(name="weights", bufs=1))
    spool = ctx.enter_context(tc.tile_pool(name="stage", bufs=1))
    xapool = ctx.enter_context(tc.tile_pool(name="xa", bufs=3))
    xbpool = ctx.enter_context(tc.tile_pool(name="xb", bufs=3))
    r1pool = ctx.enter_context(tc.tile_pool(name="relu1", bufs=2))
    epool = ctx.enter_context(tc.tile_pool(name="exp", bufs=2))
    fpool = ctx.enter_context(tc.tile_pool(name="fin", bufs=2))
    dpool = ctx.enter_context(tc.tile_pool(name="den", bufs=8))

    ps_rec = ctx.enter_context(tc.tile_pool(name="ps_rec", bufs=2, space="PSUM"))
    ps_xp = ctx.enter_context(tc.tile_pool(name="ps_xp", bufs=2, space="PSUM"))
    ps_m1 = ctx.enter_context(tc.tile_pool(name="ps_m1", bufs=2, space="PSUM"))
    ps_m2 = ctx.enter_context(tc.tile_pool(name="ps_m2", bufs=2, space="PSUM"))

    wfold_s = wpool.tile([128, 64], F32)
    nc.sync.dma_start(wfold_s[:], wfold)
    wih_a_s = wpool.tile([IH, 64], F32)
    nc.sync.dma_start(wih_a_s[:], wih_a)
    wih_b_s = wpool.tile([IH, 64], F32)
    nc.sync.dma_start(wih_b_s[:], wih_b)
    bias_s = wpool.tile([128, 1], F32)
    nc.sync.dma_start(bias_s[64:128, :], bias64)
    w1t_s = wpool.tile([64, 16], F32)
    nc.sync.dma_start(w1t_s[:], w1t)
    b1_s = wpool.tile([16, 1], F32)
    nc.sync.dma_start(b1_s[:], b1)
    w2te_s = wpool.tile([16, 4], F32)
    nc.sync.dma_start(w2te_s[:], w2te)

    stage = spool.tile([128, RB * w], F32)
    nc.vector.memset(stage[0:64, 0:w], 0.0)    # h_{-1} = 0

    q = []
    POP = 6

    def pop(k):
        for _ in range(k):
            if not q:
                return
            q.pop(0)()

    bank_state = {}

    def emit_phase1(c):
        """xp for chunk c (t-blocks [CH*c, CH*c+CH)) -> ring slot c%RC."""
        rc = (c % RC) * CW
        cw0 = c * CW
        xa = xapool.tile([IH, CW], F32)
        xb = xbpool.tile([IH, CW], F32)
        p_xp = ps_xp.tile([128, CW], F32)
        xbq = getattr(nc, XB_ENGINE)
        q.append(lambda: nc.sync.dma_start(xa[:], xt[0, :, cw0:cw0 + CW]))
        q.append(lambda: xbq.dma_start(xb[:], xt[1, :, cw0:cw0 + CW]))
        QC = 128                      # xp matmul split granularity (cols)
        for q0 in range(0, CW, QC):
            q.append(lambda q0=q0: nc.tensor.matmul(
                p_xp[64:128, q0:q0 + QC], wih_a_s[:], xa[:, q0:q0 + QC],
                start=True, stop=False))
            q.append(lambda q0=q0: nc.tensor.matmul(
                p_xp[64:128, q0:q0 + QC], wih_b_s[:], xb[:, q0:q0 + QC],
                start=False, stop=True))
        q.append(lambda: nc.vector.tensor_scalar(
            stage[64:128, rc:rc + CW], p_xp[64:128, :],
            bias_s[64:128, :], None, op0=mybir.AluOpType.add))

    def emit_phase3(c):
        """MLP for chunk c: h at ring blocks [CH*c+1, CH*c+CH] (mod RB)."""
        bk = c // CPB
        j = c % CPB
        if j == 0:
            bank_state[bk] = ps_m2.tile([128, 4 * SL * CPB], F32,
                                        name=f"p2_{bk}", tag="p2")
        p2 = bank_state[bk]
        b0 = (CH * c + 1) % RB
        p1 = ps_m1.tile([16, CW], F32)
        r1 = r1pool.tile([16, CW], F32)
        for i in range(CH):           # per-block MM1: wrap-free, fine-grained
            bi = (b0 + i) % RB
            q.append(lambda i=i, bi=bi: nc.tensor.matmul(
                p1[:, i * w:(i + 1) * w], w1t_s[:],
                stage[0:64, bi * w:(bi + 1) * w], start=True, stop=True))
        q.append(lambda: nc.vector.tensor_scalar(
            r1[:], p1[:], b1_s[:], 0.0,
            op0=mybir.AluOpType.add, op1=mybir.AluOpType.max))
        for s in range(SL):
            col = 4 * (SL * j + s)
            q.append(lambda s=s, col=col: nc.tensor.matmul(
                p2[:, col:col + 4], r1[:, 128 * s:128 * s + 128], w2te_s[:],
                start=True, stop=True))

    def emit_bank_tail(bk):
        """exp + softmax + output DMA for psum-out bank bk."""
        jn = min(CPB, NB - CPB * bk)
        p2 = bank_state[bk]
        nsl = SL * jn                       # 128-pair slices in this bank
        ncol = 4 * nsl
        e = epool.tile([128, 4 * SL * CPB], F32)
        f = fpool.tile([128, 4 * SL * CPB], F32)
        d1 = dpool.tile([128, SL * CPB], F32)
        d2 = dpool.tile([128, SL * CPB], F32)
        d3 = dpool.tile([128, SL * CPB], F32)
        r = dpool.tile([128, SL * CPB], F32)
        q.append(lambda: nc.scalar.activation(e[:, 0:ncol], p2[:, 0:ncol],
                                              AF.Exp))
        for k in range(4):
            q.append(lambda k=k: nc.vector.tensor_scalar(
                f[:, bass.ds(k, nsl, 4)], e[:, bass.ds(k, nsl, 4)],
                float(g2[k]), None, op0=mybir.AluOpType.mult))
        q.append(lambda: nc.vector.tensor_add(
            d1[:, 0:nsl], f[:, bass.ds(0, nsl, 4)], f[:, bass.ds(1, nsl, 4)]))
        q.append(lambda: nc.vector.tensor_add(
            d2[:, 0:nsl], f[:, bass.ds(2, nsl, 4)], f[:, bass.ds(3, nsl, 4)]))
        q.append(lambda: nc.vector.tensor_add(
            d3[:, 0:nsl], d1[:, 0:nsl], d2[:, 0:nsl]))
        q.append(lambda: nc.vector.reciprocal(r[:, 0:nsl], d3[:, 0:nsl]))
        for k in range(4):
            q.append(lambda k=k: nc.vector.tensor_mul(
                f[:, bass.ds(k, nsl, 4)], f[:, bass.ds(k, nsl, 4)],
                r[:, 0:nsl]))
        # f: partition p, col 4*(SL*j+s)+k ; pair q=128*s+p ; q = dt*w + b
        t0 = CPB * bk * CH
        outq = getattr(nc, OUT_ENGINE)
        if w == 128:
            # s = dt, p = b : one contiguous slab
            q.append(lambda: outq.dma_start(
                out[:, t0:t0 + CH * jn, :], f[0:128, 0:ncol]))
        elif w == 64:
            # p = (dt%2)*64 + b, s = dt//2 ; t = CH*j + 2*s + dp
            for dp in range(2):
                view = out[:, t0:t0 + CH * jn, :].rearrange(
                    "b (j r) k -> b j r k", r=CH)[:, :, bass.ds(dp, SL, 2), :]
                q.append(lambda dp=dp, view=view: outq.dma_start(
                    view, f[64 * dp:64 * dp + 64, 0:ncol]))
        else:
            raise ValueError(w)

    # ---------------- main schedule ----------------
    for c in range(min(lead, NB)):
        emit_phase1(c)
    pop(len(q))

    for t in range(T_local):
        bl = (t % RB) * w
        nbl = ((t + 1) % RB) * w
        rp = ps_rec.tile([64, w], F32)
        for g in range(G):
            nc.tensor.matmul(rp[:, GW * g:GW * g + GW], wfold_s[:],
                             stage[:, bl + GW * g:bl + GW * g + GW],
                             start=True, stop=True)
            nc.scalar.activation(stage[0:64, nbl + GW * g:nbl + GW * g + GW],
                                 rp[:, GW * g:GW * g + GW], AF.Tanh)
            pop(POP // G)
        if t % CH == CH - 1:
            c = t // CH
            if c + lead < NB:
                emit_phase1(c + lead)
            emit_phase3(c)
            if (c + 1) % CPB == 0 or c == NB - 1:
                emit_bank_tail(c // CPB)
        pop(POP - G * (POP // G))

    pop(len(q))


def build_nc(w=WIDTH, T_local=None, lead=3, g2=(1.0, 1.0, 1.0, 1.0)):
    if T_local is None:
        T_local = T // (NCORES // (B // w)) + WARM
    # Bacc.finalize() runs the sync-legalization pipeline (matmul waits ->
    # LdWeights, event semaphores) that walrus codegen requires (<=1 wait
    # per instruction).
    nc = Bacc("TRN2", target_bir_lowering=False)
    xt_d = nc.dram_tensor("xt", (2, IH, T_local * w), F32, kind="ExternalInput")
    wfold_d = nc.dram_tensor("wfold", (128, 64), F32, kind="ExternalInput")
    wih_a_d = nc.dram_tensor("wih_a", (IH, 64), F32, kind="ExternalInput")
    wih_b_d = nc.dram_tensor("wih_b", (IH, 64), F32, kind="ExternalInput")
    bias_d = nc.dram_tensor("bias64", (64, 1), F32, kind="ExternalInput")
    w1t_d = nc.dram_tensor("w1t", (64, 16), F32, kind="ExternalInput")
    b1_d = nc.dram_tensor("b1c", (16, 1), F32, kind="ExternalInput")
    w2te_d = nc.dram_tensor("w2te", (16, 4), F32, kind="ExternalInput")
    out_d = nc.dram_tensor("out", (w, T_local, 4), F32, kind="ExternalOutput")
    with tile.TileContext(nc) as tc:
        rnn_body(tc, xt_d[:], wfold_d[:], wih_a_d[:], wih_b_d[:], bias_d[:],
                 w1t_d[:], b1_d[:], w2te_d[:], out_d[:],
                 w=w, T_local=T_local, lead=lead, g2=g2)
    nc.finalize()
    return nc


def host_weights(W_ih, W_hh, b_ih, b_hh, W1, b1, W2, b2):
    f = np.float32
    return {
        "wfold": np.concatenate([W_hh.T, np.eye(64)], 0).astype(f),
        "wih_a": W_ih.T[:IH].astype(f).copy(),
        "wih_b": W_ih.T[IH:].astype(f).copy(),
        "bias64": (b_ih + b_hh).reshape(64, 1).astype(f),
        "w1t": W1.T.astype(f).copy(),
        "b1c": b1.reshape(16, 1).astype(f),
        "w2te": W2.T.astype(f).copy(),
    }


def make_xt(x_slice, w, T_local):
    """[w, T_local, I] -> [2, IH, T_local*w] (x transposed, I split in two)."""
    return np.ascontiguousarray(
        x_slice.transpose(2, 1, 0)).reshape(2, IH, T_local * w)


def kernel(**inputs):
    x = np.asarray(inputs["x"], np.float32)
    wk = host_weights(
        np.asarray(inputs["W_ih"], np.float32),
        np.asarray(inputs["W_hh"], np.float32),
        np.asarray(inputs["b_ih"], np.float32),
        np.asarray(inputs["b_hh"], np.float32),
        np.asarray(inputs["W1"], np.float32),
        np.asarray(inputs["b1"], np.float32),
        np.asarray(inputs["W2"], np.float32),
        np.asarray(inputs["b2"], np.float32),
    )
    g2 = tuple(np.exp(np.asarray(inputs["b2"], np.float64)).tolist())

    w = WIDTH
    bt = B // w
    tt = NCORES // bt
    S = T // tt
    T_local = S + WARM

    nc = build_nc(w=w, T_local=T_local, g2=g2)
    in_maps = []
    for bi in range(bt):
        for ti in range(tt):
            t0 = 0 if ti == 0 else ti * S - WARM
            xs = x[bi * w:(bi + 1) * w, t0:t0 + T_local, :]
            in_maps.append({"xt": make_xt(xs, w, T_local), **wk})

    from concourse.bass_utils import run_bass_kernel_spmd
    res = run_bass_kernel_spmd(nc, in_maps, core_ids=list(range(NCORES)),
                               trace=TRACE)
    global LAST_EXEC_NS
    LAST_EXEC_NS = res.exec_time_ns

    out = np.empty((B, T, 4), np.float32)
    ci = 0
    for bi in range(bt):
        for ti in range(tt):
            lo = 0 if ti == 0 else WARM
            out[bi * w:(bi + 1) * w, ti * S:(ti + 1) * S] = \
                np.asarray(res.results[ci]["out"])[:, lo:lo + S]
            ci += 1
    return out


# revision 19
# speedup vs baseline: 1.0123x; 1.0123x over previous
"""Trainium2 Bass kernel for nn_RNN_6296422056099 — hybrid batch x time sharding.

RNN: xp = x @ W_ih.T + b_ih + b_hh ; h_t = tanh(xp_t + h_{t-1} @ W_hh.T)
out = softmax(relu(h @ W1.T + b1) @ W2.T + b2)

8 cores = bt batch-shards x tt time-shards. Each core runs T_local = S + W
recurrence steps over batch width w = B/bt, where the first W steps are
contraction warmup (tanh RNN forgets its initial state; W=64 reaches the
fp32 noise floor). Core (bi, ti=0) feeds real x[0:T_local] starting from the
true h=0, so ALL its outputs are exact; other cores discard the first W.

Per-core layout:
  - x is pre-transposed on host to [2, 100, T_local*w] so xp matmuls consume
    it directly (no on-chip transposes).
  - stage ring buffer [128, RB*w]: per t-block of w cols, partitions 0:64
    hold h_{t-1}^T, partitions 64:128 hold xp_t^T. Recurrence step = G
    matmuls with stationary [W_hh^T ; I_64] (identity-fold adds xp in the
    PE), then ACT tanh PSUM->stage.
  - phase 1 (xp production, lead chunks ahead) and phase 3 (MLP+softmax)
    interleave into the recurrence via an emission micro-queue.
"""

import sys

for p in ("/opt/trn_rl_repo",):
    if p not in sys.path:
        sys.path.append(p)

from contextlib import ExitStack

import numpy as np

import concourse.bass as bass
import concourse.tile as tile
from concourse import mybir
from concourse._compat import with_exitstack
from concourse.bacc import Bacc

F32 = mybir.dt.float32
AF = mybir.ActivationFunctionType

B, T, I, H = 128, 2048, 200, 64
NCORES = 8
IH = 100

# ---- sharding config ----
WIDTH = 128           # batch rows per core (w); bt = B//WIDTH, tt = NCORES//bt
WARM = 32             # contraction warmup steps
G = 2                 # recurrence groups per step

TRACE = False
LAST_EXEC_NS = None

XB_ENGINE = "gpsimd"     # queue for second-half x DMA
OUT_ENGINE = "scalar"    # queue for output DMA


def _cfg(w, T_local):
    ch = 512 // w                 # t-blocks per chunk (chunk = 512 (t,b) pairs)
    assert T_local % ch == 0
    nb = T_local // ch
    sl = (ch * w) // 128          # MM2 slices per chunk (=4)
    cpb = 512 // (4 * sl)         # chunks per psum-out bank (=32)
    return ch, nb, sl, cpb


@with_exitstack
def rnn_body(ctx: ExitStack, tc: tile.TileContext,
             xt, wfold, wih_a, wih_b, bias64, w1t, b1, w2te, out,
             w: int, T_local: int, lead: int, g2=(1.0, 1.0, 1.0, 1.0)):
    nc = tc.nc
    CH, NB, SL, CPB = _cfg(w, T_local)
    GW = w // G
    RC = lead + 3                 # ring size in chunks
    RB = RC * CH                  # ring size in t-blocks
    CW = CH * w                   # cols per chunk

    wpool = ctx.enter_context(tc.tile_pool(name="weights", bufs=1))
    spool = ctx.enter_context(tc.tile_pool(name="stage", bufs=1))
    xapool = ctx.enter_context(tc.tile_pool(name="xa", bufs=3))
    xbpool = ctx.enter_context(tc.tile_pool(name="xb", bufs=3))
    r1pool = ctx.enter_context(tc.tile_pool(name="relu1", bufs=2))
    epool = ctx.enter_context(tc.tile_pool(name="exp", bufs=2))
    fpool = ctx.enter_context(tc.tile_pool(name="fin", bufs=2))
    dpool = ctx.enter_context(tc.tile_pool(name="den", bufs=8))

    ps_rec = ctx.enter_context(tc.tile_pool(name="ps_rec", bufs=2, space="PSUM"))
    ps_xp = ctx.enter_context(tc.tile_pool(name="ps_xp", bufs=2, space="PSUM"))
    ps_m1 = ctx.enter_context(tc.tile_pool(name="ps_m1", bufs=2, space="PSUM"))
    ps_m2 = ctx.enter_context(tc.tile_pool(name="ps_m2", bufs=2, space="PSUM"))

    wfold_s = wpool.tile([128, 64], F32)
    nc.sync.dma_start(wfold_s[:], wfold)
    wih_a_s = wpool.tile([IH, 64], F32)
    nc.sync.dma_start(wih_a_s[:], wih_a)
    wih_b_s = wpool.tile([IH, 64], F32)
    nc.sync.dma_start(wih_b_s[:], wih_b)
    bias_s = wpool.tile([128, 1], F32)
    nc.sync.dma_start(bias_s[64:128, :], bias64)
    w1t_s = wpool.tile([64, 16], F32)
    nc.sync.dma_start(w1t_s[:], w1t)
    b1_s = wpool.tile([16, 1], F32)
    nc.sync.dma_start(b1_s[:], b1)
    w2te_s = wpool.tile([16, 4], F32)
    nc.sync.dma_start(w2te_s[:], w2te)

    stage = spool.tile([128, RB * w], F32)
    nc.vector.memset(stage[0:64, 0:w], 0.0)    # h_{-1} = 0

    q = []
    POP = 6

    def pop(k):
        for _ in range(k):
            if not q:
                return
            q.pop(0)()

    bank_state = {}

    def emit_phase1(c):
        """xp for chunk c (t-blocks [CH*c, CH*c+CH)) -> ring slot c%RC."""
        rc = (c % RC) * CW
        cw0 = c * CW
        xa = xapool.tile([IH, CW], F32)
        xb = xbpool.tile([IH, CW], F32)
        p_xp = ps_xp.tile([128, CW], F32)
        xbq = getattr(nc, XB_ENGINE)
        q.append(lambda: nc.sync.dma_start(xa[:], xt[0, :, cw0:cw0 + CW]))
        q.append(lambda: xbq.dma_start(xb[:], xt[1, :, cw0:cw0 + CW]))
        # one weight load per input half: HW matmuls are self-loading, so
        # fine splits would re-load the 100-row stationary each piece
        q.append(lambda: nc.tensor.matmul(
            p_xp[64:128, :], wih_a_s[:], xa[:], start=True, stop=False))
        q.append(lambda: nc.tensor.matmul(
            p_xp[64:128, :], wih_b_s[:], xb[:], start=False, stop=True))
        q.append(lambda: nc.vector.tensor_scalar(
            stage[64:128, rc:rc + CW], p_xp[64:128, :],
            bias_s[64:128, :], None, op0=mybir.AluOpType.add))

    def emit_phase3(c):
        """MLP for chunk c: h at ring blocks [CH*c+1, CH*c+CH] (mod RB)."""
        bk = c // CPB
        j = c % CPB
        if j == 0:
            bank_state[bk] = ps_m2.tile([128, 4 * SL * CPB], F32,
                                        name=f"p2_{bk}", tag="p2")
        p2 = bank_state[bk]
        b0 = (CH * c + 1) % RB
        p1 = ps_m1.tile([16, CW], F32)
        r1 = r1pool.tile([16, CW], F32)
        n1 = min(CH, RB - b0)         # blocks before ring wrap
        q.append(lambda n1=n1, b0=b0: nc.tensor.matmul(
            p1[:, 0:n1 * w], w1t_s[:],
            stage[0:64, b0 * w:(b0 + n1) * w], start=True, stop=True))
        if n1 < CH:
            q.append(lambda n1=n1: nc.tensor.matmul(
                p1[:, n1 * w:CH * w], w1t_s[:],
                stage[0:64, 0:(CH - n1) * w], start=True, stop=True))
        q.append(lambda: nc.vector.tensor_scalar(
            r1[:], p1[:], b1_s[:], 0.0,
            op0=mybir.AluOpType.add, op1=mybir.AluOpType.max))
        for s in range(SL):
            col = 4 * (SL * j + s)
            q.append(lambda s=s, col=col: nc.tensor.matmul(
                p2[:, col:col + 4], r1[:, 128 * s:128 * s + 128], w2te_s[:],
                start=True, stop=True))

    def emit_bank_tail(bk):
        """exp + softmax + output DMA for psum-out bank bk."""
        jn = min(CPB, NB - CPB * bk)
        p2 = bank_state[bk]
        nsl = SL * jn                       # 128-pair slices in this bank
        ncol = 4 * nsl
        e = epool.tile([128, 4 * SL * CPB], F32)
        f = fpool.tile([128, 4 * SL * CPB], F32)
        d1 = dpool.tile([128, SL * CPB], F32)
        d2 = dpool.tile([128, SL * CPB], F32)
        d3 = dpool.tile([128, SL * CPB], F32)
        r = dpool.tile([128, SL * CPB], F32)
        q.append(lambda: nc.scalar.activation(e[:, 0:ncol], p2[:, 0:ncol],
                                              AF.Exp))
        for k in range(4):
            q.append(lambda k=k: nc.vector.tensor_scalar(
                f[:, bass.ds(k, nsl, 4)], e[:, bass.ds(k, nsl, 4)],
                float(g2[k]), None, op0=mybir.AluOpType.mult))
        q.append(lambda: nc.vector.tensor_add(
            d1[:, 0:nsl], f[:, bass.ds(0, nsl, 4)], f[:, bass.ds(1, nsl, 4)]))
        q.append(lambda: nc.vector.tensor_add(
            d2[:, 0:nsl], f[:, bass.ds(2, nsl, 4)], f[:, bass.ds(3, nsl, 4)]))
        q.append(lambda: nc.vector.tensor_add(
            d3[:, 0:nsl], d1[:, 0:nsl], d2[:, 0:nsl]))
        q.append(lambda: nc.vector.reciprocal(r[:, 0:nsl], d3[:, 0:nsl]))
        for k in range(4):
            q.append(lambda k=k: nc.vector.tensor_mul(
                f[:, bass.ds(k, nsl, 4)], f[:, bass.ds(k, nsl, 4)],
                r[:, 0:nsl]))
        # f: partition p, col 4*(SL*j+s)+k ; pair q=128*s+p ; q = dt*w + b
        t0 = CPB * bk * CH
        outq = getattr(nc, OUT_ENGINE)
        if w == 128:
            # s = dt, p = b : one contiguous slab
            q.append(lambda: outq.dma_start(
                out[:, t0:t0 + CH * jn, :], f[0:128, 0:ncol]))
        elif w == 64:
            # p = (dt%2)*64 + b, s = dt//2 ; t = CH*j + 2*s + dp
            for dp in range(2):
                view = out[:, t0:t0 + CH * jn, :].rearrange(
                    "b (j r) k -> b j r k", r=CH)[:, :, bass.ds(dp, SL, 2), :]
                q.append(lambda dp=dp, view=view: outq.dma_start(
                    view, f[64 * dp:64 * dp + 64, 0:ncol]))
        else:
            raise ValueError(w)

    # ---------------- main schedule ----------------
    for c in range(min(lead, NB)):
        emit_phase1(c)
    pop(len(q))

    for t in range(T_local):
        bl = (t % RB) * w
        nbl = ((t + 1) % RB) * w
        rp = ps_rec.tile([64, w], F32)
        for g in range(G):
            nc.tensor.matmul(rp[:, GW * g:GW * g + GW], wfold_s[:],
                             stage[:, bl + GW * g:bl + GW * g + GW],
                             start=True, stop=True)
            nc.scalar.activation(stage[0:64, nbl + GW * g:nbl + GW * g + GW],
                                 rp[:, GW * g:GW * g + GW], AF.Tanh)
            pop(POP // G)
        if t % CH == CH - 1:
            c = t // CH
            if c + lead < NB:
                emit_phase1(c + lead)
            emit_phase3(c)
            if (c + 1) % CPB == 0 or c == NB - 1:
                emit_bank_tail(c // CPB)
        pop(POP - G * (POP // G))

    pop(len(q))


def build_nc(w=WIDTH, T_local=None, lead=3, g2=(1.0, 1.0, 1.0, 1.0)):
    if T_local is None:
        T_local = T // (NCORES // (B // w)) + WARM
    # Bacc.finalize() runs the sync-legalization pipeline (matmul waits ->
    # LdWeights, event semaphores) that walrus codegen requires (<=1 wait
    # per instruction).
    nc = Bacc("TRN2", target_bir_lowering=False)
    xt_d = nc.dram_tensor("xt", (2, IH, T_local * w), F32, kind="ExternalInput")
    wfold_d = nc.dram_tensor("wfold", (128, 64), F32, kind="ExternalInput")
    wih_a_d = nc.dram_tensor("wih_a", (IH, 64), F32, kind="ExternalInput")
    wih_b_d = nc.dram_tensor("wih_b", (IH, 64), F32, kind="ExternalInput")
    bias_d = nc.dram_tensor("bias64", (64, 1), F32, kind="ExternalInput")
    w1t_d = nc.dram_tensor("w1t", (64, 16), F32, kind="ExternalInput")
    b1_d = nc.dram_tensor("b1c", (16, 1), F32, kind="ExternalInput")
    w2te_d = nc.dram_tensor("w2te", (16, 4), F32, kind="ExternalInput")
    out_d = nc.dram_tensor("out", (w, T_local, 4), F32, kind="ExternalOutput")
    with tile.TileContext(nc) as tc:
        rnn_body(tc, xt_d[:], wfold_d[:], wih_a_d[:], wih_b_d[:], bias_d[:],
                 w1t_d[:], b1_d[:], w2te_d[:], out_d[:],
                 w=w, T_local=T_local, lead=lead, g2=g2)
    nc.finalize()
    return nc


def host_weights(W_ih, W_hh, b_ih, b_hh, W1, b1, W2, b2):
    f = np.float32
    return {
        "wfold": np.concatenate([W_hh.T, np.eye(64)], 0).astype(f),
        "wih_a": W_ih.T[:IH].astype(f).copy(),
        "wih_b": W_ih.T[IH:].astype(f).copy(),
        "bias64": (b_ih + b_hh).reshape(64, 1).astype(f),
        "w1t": W1.T.astype(f).copy(),
        "b1c": b1.reshape(16, 1).astype(f),
        "w2te": W2.T.astype(f).copy(),
    }


def make_xt(x_slice, w, T_local):
    """[w, T_local, I] -> [2, IH, T_local*w] (x transposed, I split in two)."""
    return np.ascontiguousarray(
        x_slice.transpose(2, 1, 0)).reshape(2, IH, T_local * w)


def kernel(**inputs):
    x = np.asarray(inputs["x"], np.float32)
    wk = host_weights(
        np.asarray(inputs["W_ih"], np.float32),
        np.asarray(inputs["W_hh"], np.float32),
        np.asarray(inputs["b_ih"], np.float32),
        np.asarray(inputs["b_hh"], np.float32),
        np.asarray(inputs["W1"], np.float32),
        np.asarray(inputs["b1"], np.float32),
        np.asarray(inputs["W2"], np.float32),
        np.asarray(inputs["b2"], np.float32),
    )
    g2 = tuple(np.exp(np.asarray(inputs["b2"], np.float64)).tolist())

    w = WIDTH
    bt = B // w
    tt = NCORES // bt
    S = T // tt
    T_local = S + WARM

    nc = build_nc(w=w, T_local=T_local, g2=g2)
    in_maps = []
    for bi in range(bt):
        for ti in range(tt):
            t0 = 0 if ti == 0 else ti * S - WARM
            xs = x[bi * w:(bi + 1) * w, t0:t0 + T_local, :]
            in_maps.append({"xt": make_xt(xs, w, T_local), **wk})

    from concourse.bass_utils import run_bass_kernel_spmd
    res = run_bass_kernel_spmd(nc, in_maps, core_ids=list(range(NCORES)),
                               trace=TRACE)
    global LAST_EXEC_NS
    LAST_EXEC_NS = res.exec_time_ns

    out = np.empty((B, T, 4), np.float32)
    ci = 0
    for bi in range(bt):
        for ti in range(tt):
            lo = 0 if ti == 0 else WARM
            out[bi * w:(bi + 1) * w, ti * S:(ti + 1) * S] = \
                np.asarray(res.results[ci]["out"])[:, lo:lo + S]
            ci += 1
    return out


# revision 20
# speedup vs baseline: 1.1812x; 1.1668x over previous
"""Trainium2 Bass kernel for nn_RNN_6296422056099 — hybrid batch x time sharding.

RNN: xp = x @ W_ih.T + b_ih + b_hh ; h_t = tanh(xp_t + h_{t-1} @ W_hh.T)
out = softmax(relu(h @ W1.T + b1) @ W2.T + b2)

8 cores = bt batch-shards x tt time-shards. Each core runs T_local = S + W
recurrence steps over batch width w = B/bt, where the first W steps are
contraction warmup (tanh RNN forgets its initial state; W=64 reaches the
fp32 noise floor). Core (bi, ti=0) feeds real x[0:T_local] starting from the
true h=0, so ALL its outputs are exact; other cores discard the first W.

Per-core layout:
  - x is pre-transposed on host to [2, 100, T_local*w] so xp matmuls consume
    it directly (no on-chip transposes).
  - stage ring buffer [128, RB*w]: per t-block of w cols, partitions 0:64
    hold h_{t-1}^T, partitions 64:128 hold xp_t^T. Recurrence step = G
    matmuls with stationary [W_hh^T ; I_64] (identity-fold adds xp in the
    PE), then ACT tanh PSUM->stage.
  - phase 1 (xp production, lead chunks ahead) and phase 3 (MLP+softmax)
    interleave into the recurrence via an emission micro-queue.
"""

import sys

for p in ("/opt/trn_rl_repo",):
    if p not in sys.path:
        sys.path.append(p)

from contextlib import ExitStack

import numpy as np

import concourse.bass as bass
import concourse.tile as tile
from concourse import mybir
from concourse._compat import with_exitstack
from concourse.bacc import Bacc

F32 = mybir.dt.float32
AF = mybir.ActivationFunctionType

B, T, I, H = 128, 2048, 200, 64
NCORES = 8
IH = 100

# ---- sharding config ----
WIDTH = 128           # batch rows per core (w); bt = B//WIDTH, tt = NCORES//bt
WARM = 32             # contraction warmup steps
G = 1                 # recurrence groups per step (1 = fewest weight reloads)

TRACE = False
LAST_EXEC_NS = None

XB_ENGINE = "gpsimd"     # queue for second-half x DMA
OUT_ENGINE = "scalar"    # queue for output DMA


def _cfg(w, T_local):
    ch = 512 // w                 # t-blocks per chunk (chunk = 512 (t,b) pairs)
    assert T_local % ch == 0
    nb = T_local // ch
    sl = (ch * w) // 128          # MM2 slices per chunk (=4)
    cpb = 512 // (4 * sl)         # chunks per psum-out bank (=32)
    return ch, nb, sl, cpb


@with_exitstack
def rnn_body(ctx: ExitStack, tc: tile.TileContext,
             xt, wfold, wih_a, wih_b, bias64, w1t, b1, w2te, out,
             w: int, T_local: int, lead: int, g2=(1.0, 1.0, 1.0, 1.0)):
    nc = tc.nc
    CH, NB, SL, CPB = _cfg(w, T_local)
    GW = w // G
    RC = lead + 3                 # ring size in chunks
    RB = RC * CH                  # ring size in t-blocks
    CW = CH * w                   # cols per chunk

    wpool = ctx.enter_context(tc.tile_pool(name="weights", bufs=1))
    spool = ctx.enter_context(tc.tile_pool(name="stage", bufs=1))
    xapool = ctx.enter_context(tc.tile_pool(name="xa", bufs=3))
    xbpool = ctx.enter_context(tc.tile_pool(name="xb", bufs=3))
    r1pool = ctx.enter_context(tc.tile_pool(name="relu1", bufs=2))
    epool = ctx.enter_context(tc.tile_pool(name="exp", bufs=2))
    fpool = ctx.enter_context(tc.tile_pool(name="fin", bufs=2))
    dpool = ctx.enter_context(tc.tile_pool(name="den", bufs=8))

    ps_rec = ctx.enter_context(tc.tile_pool(name="ps_rec", bufs=2, space="PSUM"))
    ps_xp = ctx.enter_context(tc.tile_pool(name="ps_xp", bufs=2, space="PSUM"))
    ps_m1 = ctx.enter_context(tc.tile_pool(name="ps_m1", bufs=2, space="PSUM"))
    ps_m2 = ctx.enter_context(tc.tile_pool(name="ps_m2", bufs=2, space="PSUM"))

    wfold_s = wpool.tile([128, 64], F32)
    nc.sync.dma_start(wfold_s[:], wfold)
    wih_a_s = wpool.tile([IH, 64], F32)
    nc.sync.dma_start(wih_a_s[:], wih_a)
    wih_b_s = wpool.tile([IH, 64], F32)
    nc.sync.dma_start(wih_b_s[:], wih_b)
    bias_s = wpool.tile([128, 1], F32)
    nc.sync.dma_start(bias_s[64:128, :], bias64)
    w1t_s = wpool.tile([64, 16], F32)
    nc.sync.dma_start(w1t_s[:], w1t)
    b1_s = wpool.tile([16, 1], F32)
    nc.sync.dma_start(b1_s[:], b1)
    w2te_s = wpool.tile([16, 4], F32)
    nc.sync.dma_start(w2te_s[:], w2te)

    stage = spool.tile([128, RB * w], F32)
    nc.vector.memset(stage[0:64, 0:w], 0.0)    # h_{-1} = 0

    q = []
    POP = 6

    def pop(k):
        for _ in range(k):
            if not q:
                return
            q.pop(0)()

    bank_state = {}

    def emit_phase1(c):
        """xp for chunk c (t-blocks [CH*c, CH*c+CH)) -> ring slot c%RC."""
        rc = (c % RC) * CW
        cw0 = c * CW
        xa = xapool.tile([IH, CW], F32)
        xb = xbpool.tile([IH, CW], F32)
        p_xp = ps_xp.tile([128, CW], F32)
        xbq = getattr(nc, XB_ENGINE)
        q.append(lambda: nc.sync.dma_start(xa[:], xt[0, :, cw0:cw0 + CW]))
        q.append(lambda: xbq.dma_start(xb[:], xt[1, :, cw0:cw0 + CW]))
        # one weight load per input half: HW matmuls are self-loading, so
        # fine splits would re-load the 100-row stationary each piece
        q.append(lambda: nc.tensor.matmul(
            p_xp[64:128, :], wih_a_s[:], xa[:], start=True, stop=False))
        q.append(lambda: nc.tensor.matmul(
            p_xp[64:128, :], wih_b_s[:], xb[:], start=False, stop=True))
        q.append(lambda: nc.vector.tensor_scalar(
            stage[64:128, rc:rc + CW], p_xp[64:128, :],
            bias_s[64:128, :], None, op0=mybir.AluOpType.add))

    def emit_phase3(c):
        """MLP for chunk c: h at ring blocks [CH*c+1, CH*c+CH] (mod RB)."""
        bk = c // CPB
        j = c % CPB
        if j == 0:
            bank_state[bk] = ps_m2.tile([128, 4 * SL * CPB], F32,
                                        name=f"p2_{bk}", tag="p2")
        p2 = bank_state[bk]
        b0 = (CH * c + 1) % RB
        p1 = ps_m1.tile([16, CW], F32)
        r1 = r1pool.tile([16, CW], F32)
        n1 = min(CH, RB - b0)         # blocks before ring wrap
        q.append(lambda n1=n1, b0=b0: nc.tensor.matmul(
            p1[:, 0:n1 * w], w1t_s[:],
            stage[0:64, b0 * w:(b0 + n1) * w], start=True, stop=True))
        if n1 < CH:
            q.append(lambda n1=n1: nc.tensor.matmul(
                p1[:, n1 * w:CH * w], w1t_s[:],
                stage[0:64, 0:(CH - n1) * w], start=True, stop=True))
        q.append(lambda: nc.vector.tensor_scalar(
            r1[:], p1[:], b1_s[:], 0.0,
            op0=mybir.AluOpType.add, op1=mybir.AluOpType.max))
        for s in range(SL):
            col = 4 * (SL * j + s)
            q.append(lambda s=s, col=col: nc.tensor.matmul(
                p2[:, col:col + 4], r1[:, 128 * s:128 * s + 128], w2te_s[:],
                start=True, stop=True))

    def emit_bank_tail(bk):
        """exp + softmax + output DMA for psum-out bank bk."""
        jn = min(CPB, NB - CPB * bk)
        p2 = bank_state[bk]
        nsl = SL * jn                       # 128-pair slices in this bank
        ncol = 4 * nsl
        e = epool.tile([128, 4 * SL * CPB], F32)
        f = fpool.tile([128, 4 * SL * CPB], F32)
        d1 = dpool.tile([128, SL * CPB], F32)
        d2 = dpool.tile([128, SL * CPB], F32)
        d3 = dpool.tile([128, SL * CPB], F32)
        r = dpool.tile([128, SL * CPB], F32)
        q.append(lambda: nc.scalar.activation(e[:, 0:ncol], p2[:, 0:ncol],
                                              AF.Exp))
        for k in range(4):
            q.append(lambda k=k: nc.vector.tensor_scalar(
                f[:, bass.ds(k, nsl, 4)], e[:, bass.ds(k, nsl, 4)],
                float(g2[k]), None, op0=mybir.AluOpType.mult))
        q.append(lambda: nc.vector.tensor_add(
            d1[:, 0:nsl], f[:, bass.ds(0, nsl, 4)], f[:, bass.ds(1, nsl, 4)]))
        q.append(lambda: nc.vector.tensor_add(
            d2[:, 0:nsl], f[:, bass.ds(2, nsl, 4)], f[:, bass.ds(3, nsl, 4)]))
        q.append(lambda: nc.vector.tensor_add(
            d3[:, 0:nsl], d1[:, 0:nsl], d2[:, 0:nsl]))
        q.append(lambda: nc.vector.reciprocal(r[:, 0:nsl], d3[:, 0:nsl]))
        for k in range(4):
            q.append(lambda k=k: nc.vector.tensor_mul(
                f[:, bass.ds(k, nsl, 4)], f[:, bass.ds(k, nsl, 4)],
                r[:, 0:nsl]))
        # f: partition p, col 4*(SL*j+s)+k ; pair q=128*s+p ; q = dt*w + b
        t0 = CPB * bk * CH
        outq = getattr(nc, OUT_ENGINE)
        if w == 128:
            # s = dt, p = b : one contiguous slab
            q.append(lambda: outq.dma_start(
                out[:, t0:t0 + CH * jn, :], f[0:128, 0:ncol]))
        elif w == 64:
            # p = (dt%2)*64 + b, s = dt//2 ; t = CH*j + 2*s + dp
            for dp in range(2):
                view = out[:, t0:t0 + CH * jn, :].rearrange(
                    "b (j r) k -> b j r k", r=CH)[:, :, bass.ds(dp, SL, 2), :]
                q.append(lambda dp=dp, view=view: outq.dma_start(
                    view, f[64 * dp:64 * dp + 64, 0:ncol]))
        else:
            raise ValueError(w)

    # ---------------- main schedule ----------------
    for c in range(min(lead, NB)):
        emit_phase1(c)
    pop(len(q))

    for t in range(T_local):
        bl = (t % RB) * w
        nbl = ((t + 1) % RB) * w
        rp = ps_rec.tile([64, w], F32)
        for g in range(G):
            nc.tensor.matmul(rp[:, GW * g:GW * g + GW], wfold_s[:],
                             stage[:, bl + GW * g:bl + GW * g + GW],
                             start=True, stop=True)
            nc.scalar.activation(stage[0:64, nbl + GW * g:nbl + GW * g + GW],
                                 rp[:, GW * g:GW * g + GW], AF.Tanh)
            pop(POP // G)
        if t % CH == CH - 1:
            c = t // CH
            if c + lead < NB:
                emit_phase1(c + lead)
            emit_phase3(c)
            if (c + 1) % CPB == 0 or c == NB - 1:
                emit_bank_tail(c // CPB)
        pop(POP - G * (POP // G))

    pop(len(q))


def build_nc(w=WIDTH, T_local=None, lead=3, g2=(1.0, 1.0, 1.0, 1.0)):
    if T_local is None:
        T_local = T // (NCORES // (B // w)) + WARM
    # Bacc.finalize() runs the sync-legalization pipeline (matmul waits ->
    # LdWeights, event semaphores) that walrus codegen requires (<=1 wait
    # per instruction).
    nc = Bacc("TRN2", target_bir_lowering=False)
    xt_d = nc.dram_tensor("xt", (2, IH, T_local * w), F32, kind="ExternalInput")
    wfold_d = nc.dram_tensor("wfold", (128, 64), F32, kind="ExternalInput")
    wih_a_d = nc.dram_tensor("wih_a", (IH, 64), F32, kind="ExternalInput")
    wih_b_d = nc.dram_tensor("wih_b", (IH, 64), F32, kind="ExternalInput")
    bias_d = nc.dram_tensor("bias64", (64, 1), F32, kind="ExternalInput")
    w1t_d = nc.dram_tensor("w1t", (64, 16), F32, kind="ExternalInput")
    b1_d = nc.dram_tensor("b1c", (16, 1), F32, kind="ExternalInput")
    w2te_d = nc.dram_tensor("w2te", (16, 4), F32, kind="ExternalInput")
    out_d = nc.dram_tensor("out", (w, T_local, 4), F32, kind="ExternalOutput")
    with tile.TileContext(nc) as tc:
        rnn_body(tc, xt_d[:], wfold_d[:], wih_a_d[:], wih_b_d[:], bias_d[:],
                 w1t_d[:], b1_d[:], w2te_d[:], out_d[:],
                 w=w, T_local=T_local, lead=lead, g2=g2)
    nc.finalize()
    return nc


def host_weights(W_ih, W_hh, b_ih, b_hh, W1, b1, W2, b2):
    f = np.float32
    return {
        "wfold": np.concatenate([W_hh.T, np.eye(64)], 0).astype(f),
        "wih_a": W_ih.T[:IH].astype(f).copy(),
        "wih_b": W_ih.T[IH:].astype(f).copy(),
        "bias64": (b_ih + b_hh).reshape(64, 1).astype(f),
        "w1t": W1.T.astype(f).copy(),
        "b1c": b1.reshape(16, 1).astype(f),
        "w2te": W2.T.astype(f).copy(),
    }


def make_xt(x_slice, w, T_local):
    """[w, T_local, I] -> [2, IH, T_local*w] (x transposed, I split in two)."""
    return np.ascontiguousarray(
        x_slice.transpose(2, 1, 0)).reshape(2, IH, T_local * w)


def kernel(**inputs):
    x = np.asarray(inputs["x"], np.float32)
    wk = host_weights(
        np.asarray(inputs["W_ih"], np.float32),
        np.asarray(inputs["W_hh"], np.float32),
        np.asarray(inputs["b_ih"], np.float32),
        np.asarray(inputs["b_hh"], np.float32),
        np.asarray(inputs["W1"], np.float32),
        np.asarray(inputs["b1"], np.float32),
        np.asarray(inputs["W2"], np.float32),
        np.asarray(inputs["b2"], np.float32),
    )
    g2 = tuple(np.exp(np.asarray(inputs["b2"], np.float64)).tolist())

    w = WIDTH
    bt = B // w
    tt = NCORES // bt
    S = T // tt
    T_local = S + WARM

    nc = build_nc(w=w, T_local=T_local, g2=g2)
    in_maps = []
    for bi in range(bt):
        for ti in range(tt):
            t0 = 0 if ti == 0 else ti * S - WARM
            xs = x[bi * w:(bi + 1) * w, t0:t0 + T_local, :]
            in_maps.append({"xt": make_xt(xs, w, T_local), **wk})

    from concourse.bass_utils import run_bass_kernel_spmd
    res = run_bass_kernel_spmd(nc, in_maps, core_ids=list(range(NCORES)),
                               trace=TRACE)
    global LAST_EXEC_NS
    LAST_EXEC_NS = res.exec_time_ns

    out = np.empty((B, T, 4), np.float32)
    ci = 0
    for bi in range(bt):
        for ti in range(tt):
            lo = 0 if ti == 0 else WARM
            out[bi * w:(bi + 1) * w, ti * S:(ti + 1) * S] = \
                np.asarray(res.results[ci]["out"])[:, lo:lo + S]
            ci += 1
    return out
